# revision 26
# baseline (speedup 1.0000x reference)
"""Adaptive average pooling (16,250,250,256) -> (16,7,7,256), NHWC, f32.

Sharding: data-parallel over batch — 2 images per NeuronCore, 8 cores,
no collectives; host concatenates the per-core outputs.

Per-core algorithm (memory-bound; built around DMA efficiency):
  - x tiles: H on partitions, 46 w-columns per chunk -> one 46KB
    contiguous DRAM run per partition = ONE descriptor per partition
    (the DGE splits descriptors at a 50KB quantum). The two h-chunks'
    loads are issued on DIFFERENT descriptor generators - sync (HWDGE)
    and gpsimd (SWDGE) - which both pace at ~145ns/descriptor, so the
    two streams together outrun HBM (~358GB/s/core).
  - Both pooling axes happen on the TensorEngine via PSUM accumulation:
    for each w column, one fp32r matmul per h-chunk with a [h,7] 0/1
    h-bin-indicator weight matrix accumulates into the PSUM slab of
    that column's w-bin (fp32r streams 1 row/cycle at N=256).
    7 slabs = 7 PSUM banks; w-bin overlaps just issue two matmuls.
  - Epilogue on ScalarE: activation-copy each slab scaled by
    1/(count_h[i]*count_w[j]); one contiguous output DMA per batch.
  - VectorE does nothing; GpSimd runs only dma_starts.
"""

import sys

for _p in ("/opt/trn_rl_repo",):
    if _p not in sys.path:
        sys.path.insert(0, _p)

import numpy as np

from concourse import bacc, mybir, tile
from concourse.bass_utils import run_bass_kernel_spmd

B, H, W, C = 16, 250, 250, 256
OUT_H = OUT_W = 7
NCORES = 8
BPC = B // NCORES  # batches per core

NW_DMA = 46  # w columns per DMA chunk (46KB f32 per partition run)


def _bin_edges(in_size, out_size):
    scale = np.float32(in_size / out_size)
    idx = np.arange(out_size, dtype=np.float32)
    starts = (idx * scale).astype(np.int32)
    ends = np.ceil((idx + 1.0) * scale).astype(np.int32)
    return starts, ends


SX, EX = _bin_edges(H, OUT_H)
SY, EY = _bin_edges(W, OUT_W)
CH = EX - SX
CW = EY - SY

# Both h-chunks are exactly 128 partitions (the DGEs only spread a DMA
# across all 16 SDMA engines for 128-partition transfers). They overlap
# in rows 122..127; the second chunk's weights are zero there.
HCHUNKS = [(0, 128), (122, 128)]
HEFF = [(0, 128), (128, 250)]  # effective (non-duplicated) row ranges
WCHUNKS_DMA = [(i * NW_DMA, min(NW_DMA, W - i * NW_DMA))
               for i in range((W + NW_DMA - 1) // NW_DMA)]

_NC_CACHE = []


def _build():
    nc = bacc.Bacc("TRN2", target_bir_lowering=False, debug=False,
                   num_devices=NCORES)
    f32 = mybir.dt.float32
    f32r = mybir.dt.float32r
    x = nc.dram_tensor("x", [BPC, H, W, C], f32r, kind="ExternalInput").ap()
    pt = nc.dram_tensor("pt", [2, 128, OUT_H], f32r,
                        kind="ExternalInput").ap()
    sc = nc.dram_tensor("sc", [OUT_H, OUT_W], f32,
                        kind="ExternalInput").ap()
    out = nc.dram_tensor("out", [BPC, OUT_H, OUT_W, C], f32,
                         kind="ExternalOutput").ap()

    with tile.TileContext(nc) as tc:
        with tc.tile_pool(name="const", bufs=1) as cpool, \
             tc.tile_pool(name="xp", bufs=2) as xpool, \
             tc.tile_pool(name="op", bufs=2) as opool, \
             tc.tile_pool(name="ps", bufs=1, space="PSUM") as pspool:
            ptts = []
            for hci, (h0, hp) in enumerate(HCHUNKS):
                ptt = cpool.tile([hp, OUT_H], f32r, name=f"pt{hci}")
                nc.scalar.dma_start(ptt[:], pt[hci, 0:hp, :])
                ptts.append(ptt)
            sc_t = cpool.tile([OUT_H, OUT_W], f32, name="sc_t")
            nc.scalar.dma_start(sc_t[:], sc[:])

            for b in range(BPC):
                slabs = [pspool.tile([OUT_H, C], f32, tag=f"sl{j}",
                                     name=f"sl{j}_{b}")
                         for j in range(OUT_W)]
                for (dw0, dnw) in WCHUNKS_DMA:
                    xts = []
                    for hci, (h0, hp) in enumerate(HCHUNKS):
                        xt = xpool.tile([hp, dnw * C], f32r, tag=f"x{hci}",
                                        name=f"x{hci}_{b}_{dw0}")
                        src = x[b, h0:h0 + hp, dw0:dw0 + dnw, :]
                        src = src.rearrange("h w c -> h (w c)")
                        eng = nc.sync if hci == 0 else nc.gpsimd
                        eng.dma_start(xt[:], src)
                        xts.append(xt)
                    for hci in range(2):
                        for wl in range(dnw):
                            w = dw0 + wl
                            rhs = xts[hci][:, wl * C:(wl + 1) * C]
                            for j in range(OUT_W):
                                if not (SY[j] <= w < EY[j]):
                                    continue
                                nc.tensor.matmul(
                                    slabs[j][:], ptts[hci][:], rhs,
                                    start=(w == SY[j] and hci == 0),
                                    stop=(w == EY[j] - 1 and hci == 1))
                osb = opool.tile([OUT_H, OUT_W * C], f32, tag="osb",
                                 name=f"osb{b}")
                for j in range(OUT_W):
                    nc.scalar.mul(osb[:, j * C:(j + 1) * C], slabs[j][:],
                                  sc_t[:, j:j + 1])
                nc.scalar.dma_start(
                    out[b], osb.rearrange("i (j c) -> i j c", c=C))

    nc.compile()
    return nc


def _get_nc():
    if not _NC_CACHE:
        _NC_CACHE.append(_build())
    return _NC_CACHE[0]


def _consts_np():
    ptv = np.zeros((2, 128, OUT_H), dtype=np.float32)
    for hci, (h0, hp) in enumerate(HCHUNKS):
        e0, e1 = HEFF[hci]
        for p in range(hp):
            h = h0 + p
            if not (e0 <= h < e1):
                continue
            for i in range(OUT_H):
                if SX[i] <= h < EX[i]:
                    ptv[hci, p, i] = 1.0
    scv = (1.0 / (CH.astype(np.float32)[:, None]
                  * CW.astype(np.float32)[None, :]))
    return ptv, scv.astype(np.float32)


def run(x: np.ndarray, **spmd_kwargs):
    x = np.ascontiguousarray(x, dtype=np.float32)
    assert x.shape == (B, H, W, C), x.shape
    nc = _get_nc()
    ptv, scv = _consts_np()
    in_maps = [{"x": x[i * BPC:(i + 1) * BPC], "pt": ptv, "sc": scv}
               for i in range(NCORES)]
    res = run_bass_kernel_spmd(nc, in_maps, core_ids=list(range(NCORES)),
                               **spmd_kwargs)
    out = np.concatenate([res.results[i]["out"] for i in range(NCORES)],
                         axis=0)
    return out, res


def kernel(x: np.ndarray) -> np.ndarray:
    out, _ = run(x)
    return out
